# revision 14
# baseline (speedup 1.0000x reference)
"""Trainium2 Bass kernel for nn_BranchRoute (threshold MoE routing).

reference:
    score = sigmoid(x @ W_gate + b_gate)          # [N, 2]
    hot   = score > 0.5                           # == (x @ W_gate + b_gate) > 0
    x_0   = where(hot[:, 0:1], x, 0)
    x_1   = where(hot[:, 1:2], x, 0)
    x_comb = x_0 + x_1

Sharding: data-parallel over tokens across 8 NeuronCores (2048 tokens/core),
gate weights replicated.  Per core the kernel streams 16 tiles of
[128 tokens, 1024 d]: gate logits via fused multiply+reduce on DVE
(sigmoid(z) > 0.5  <=>  z > -b, so no sigmoid is evaluated), then three
per-partition-scalar mask multiplies spread across ACT / POOL / DVE, and
three output DMAs.  The kernel is DMA-bound (8 MiB in + 24 MiB out per core).
"""

import numpy as np

N_TOKENS = 16384
D_MODEL = 1024
N_BRANCHES = 2
N_CORES = 8
N_SHARD = N_TOKENS // N_CORES  # 2048 tokens per core
P = 128                        # SBUF partitions
NTILES = N_SHARD // P          # 16 token-tiles per core

_CACHE = {}


def _split_multi_waits(nc, max_embedded=1):
    """This container's walrus build rejects instructions carrying more than
    one embedded semaphore wait ("Too many sync wait commands").  Hoist the
    extra waits into standalone EventSemaphore instructions immediately
    before the owning instruction on the same engine — identical ordering
    semantics, encodable by this compiler."""
    from concourse import mybir

    wid = 0
    for fn in nc.m.functions:
        for bb in fn.blocks:
            out = []
            changed = False
            for inst in bb.instructions:
                si = getattr(inst, "sync_info", None)
                waits = list(si.on_wait) if si is not None else []
                if si is not None and len(waits) > max_embedded:
                    extra, keep = waits[:-max_embedded], waits[-max_embedded:]
                    for w in extra:
                        es = mybir.InstEventSemaphore(
                            name=f"WSPLIT-{wid}", ins=[], outs=[]
                        )
                        wid += 1
                        es.engine = inst.engine
                        es.sync_info = mybir.SyncInfo(on_wait=[w], on_update=[])
                        out.append(es)
                    si.on_wait = keep
                    changed = True
                out.append(inst)
            if changed:
                bb.instructions = out


def _build_bass():
    import concourse.bass as bass
    import concourse.tile as tile
    from concourse import mybir

    f32 = mybir.dt.float32
    nc = bass.Bass(trn_type="TRN2")

    # w is passed host-transposed as [N_BRANCHES, D_MODEL] so the partition
    # broadcast DMA below reads contiguous 4 KiB rows instead of stride-2
    # element gathers.
    x_h = nc.dram_tensor("x", [N_SHARD, D_MODEL], f32, kind="ExternalInput")
    w_h = nc.dram_tensor("w", [N_BRANCHES, D_MODEL], f32, kind="ExternalInput")
    b_h = nc.dram_tensor("b", [1, N_BRANCHES], f32, kind="ExternalInput")
    o0_h = nc.dram_tensor("o0", [N_SHARD, D_MODEL], f32, kind="ExternalOutput")
    o1_h = nc.dram_tensor("o1", [N_SHARD, D_MODEL], f32, kind="ExternalOutput")
    oc_h = nc.dram_tensor("oc", [N_SHARD, D_MODEL], f32, kind="ExternalOutput")

    # Pair token-tiles: [npair, 128, 2, 1024] — one 1 MiB DMA per pair,
    # partition dim leading on both sides so the DMA APs balance.
    TB = 2
    NPAIR = NTILES // TB
    x_t = x_h[:].rearrange("(t s p) d -> t p s d", s=TB, p=P)
    o0_t = o0_h[:].rearrange("(t s p) d -> t p s d", s=TB, p=P)
    o1_t = o1_h[:].rearrange("(t s p) d -> t p s d", s=TB, p=P)
    oc_t = oc_h[:].rearrange("(t s p) d -> t p s d", s=TB, p=P)

    with tile.TileContext(nc) as tc:
        with (
            tc.tile_pool(name="singles", bufs=1) as singles,
            tc.tile_pool(name="xp", bufs=4) as xp,
            tc.tile_pool(name="out0", bufs=4) as p0,
            tc.tile_pool(name="out1", bufs=4) as p1,
            tc.tile_pool(name="outc", bufs=4) as pc,
            tc.tile_pool(name="small", bufs=8) as small,
        ):
            # W rows broadcast across all 128 partitions:
            # wb[p, br*D : (br+1)*D] = W^T[br, :]
            wb = singles.tile([P, N_BRANCHES * D_MODEL], f32)
            w_ap = w_h[:]
            w_bcast = bass.AP(
                tensor=w_ap.tensor,
                offset=w_ap.offset,
                ap=[[0, P], [1, N_BRANCHES * D_MODEL]],
            )
            nc.sync.dma_start(out=wb, in_=w_bcast)

            # negb[p, br] = -b[br], broadcast across partitions
            negb = singles.tile([P, N_BRANCHES], f32)
            b_ap = b_h[:]
            b_bcast = bass.AP(
                tensor=b_ap.tensor,
                offset=b_ap.offset,
                ap=[[0, P], b_ap.ap[1]],
            )
            nc.sync.dma_start(out=negb, in_=b_bcast)
            nc.vector.tensor_scalar_mul(out=negb, in0=negb, scalar1=-1.0)

            for i in range(NPAIR):
                x_sb = xp.tile([P, TB, D_MODEL], f32)
                nc.sync.dma_start(out=x_sb, in_=x_t[i])

                o0 = p0.tile([P, TB, D_MODEL], f32)
                o1 = p1.tile([P, TB, D_MODEL], f32)
                oc = pc.tile([P, TB, D_MODEL], f32)

                for s in range(TB):
                    x_s = x_sb[:, s, :]

                    # z[p, br] = sum_d x[p, d] * W[d, br]  (fused DVE pass/branch)
                    z = small.tile([P, N_BRANCHES], f32)
                    for br in range(N_BRANCHES):
                        scratch = xp.tile([P, D_MODEL], f32, tag="scratch")
                        nc.vector.scalar_tensor_tensor(
                            out=scratch,
                            in0=x_s,
                            scalar=0.0,
                            in1=wb[:, br * D_MODEL : (br + 1) * D_MODEL],
                            op0=mybir.AluOpType.bypass,
                            op1=mybir.AluOpType.mult,
                            accum_out=z[:, br : br + 1],
                        )

                    # hot mask: m = (z > -b) as 1.0/0.0 ; mc = m0 + m1
                    m = small.tile([P, N_BRANCHES], f32)
                    nc.vector.tensor_tensor(
                        out=m, in0=z, in1=negb, op=mybir.AluOpType.is_gt
                    )
                    mc = small.tile([P, 1], f32)
                    nc.vector.tensor_add(out=mc, in0=m[:, 0:1], in1=m[:, 1:2])

                    # masked outputs: x * m (per-partition scalar broadcast);
                    # xc alternates DVE/ACT to balance the two engines
                    nc.scalar.mul(out=o0[:, s, :], in_=x_s, mul=m[:, 0:1])
                    nc.scalar.mul(out=o1[:, s, :], in_=x_s, mul=m[:, 1:2])
                    if s % 2 == 0:
                        nc.vector.tensor_scalar_mul(
                            out=oc[:, s, :], in0=x_s, scalar1=mc
                        )
                    else:
                        nc.scalar.mul(out=oc[:, s, :], in_=x_s, mul=mc)

                # Spread stores over three DMA paths: o0 on the ACT HWDGE
                # queue, o1 + x loads on the SP HWDGE queue, oc on the Pool
                # SWDGE queue.
                nc.scalar.dma_start(out=o0_t[i], in_=o0)
                nc.sync.dma_start(out=o1_t[i], in_=o1)
                nc.gpsimd.dma_start(out=oc_t[i], in_=oc)

    _split_multi_waits(nc)
    return nc


def _get_nc():
    if "nc" not in _CACHE:
        _CACHE["nc"] = _build_bass()
    return _CACHE["nc"]


LAST_EXEC_NS = None
LAST_TRACE = None


def kernel(x, W_gate, b_gate, _trace=False):
    global LAST_EXEC_NS, LAST_TRACE
    from concourse.bass_utils import run_bass_kernel_spmd

    x = np.ascontiguousarray(np.asarray(x, dtype=np.float32))
    w = np.ascontiguousarray(np.asarray(W_gate, dtype=np.float32).T)  # [NB, D]
    b = np.ascontiguousarray(np.asarray(b_gate, dtype=np.float32)).reshape(
        1, N_BRANCHES
    )

    nc = _get_nc()
    in_maps = [
        {"x": x[c * N_SHARD : (c + 1) * N_SHARD], "w": w, "b": b}
        for c in range(N_CORES)
    ]
    res = run_bass_kernel_spmd(
        nc, in_maps, core_ids=list(range(N_CORES)), trace=_trace
    )
    LAST_EXEC_NS = res.exec_time_ns
    LAST_TRACE = getattr(res, "instructions_and_trace", None)

    x0 = np.concatenate([res.results[c]["o0"] for c in range(N_CORES)], axis=0)
    x1 = np.concatenate([res.results[c]["o1"] for c in range(N_CORES)], axis=0)
    xc = np.concatenate([res.results[c]["oc"] for c in range(N_CORES)], axis=0)
    return (x0, x1, xc)
